# revision 8
# baseline (speedup 1.0000x reference)
import numpy as np
import concourse.bass as bass
import concourse.bacc as bacc
import concourse.mybir as mybir
import concourse.tile as tile
from concourse.bass_utils import run_bass_kernel_spmd

# Problem: B=16384 rows, each row is a 1024-point complex FFT expressed as a
# fixed linear map on 2048 interleaved [Re,Im] floats. The whole reference
# pipeline (bit-reversal, butterfly stages, unscramble) is linear, so
# out_row = M @ in_row for a fixed 2048x2048 fp32 matrix M built on host.
# Kernel: pure data-parallel over 8 cores, each does y = x_shard @ M^T via
# TensorE (float32r fast path), with PE transposes to get the contraction
# dim onto partitions.

B = 16384
W = 2048          # row width (2*N complex interleaved)
NCORES = 8
RPC = B // NCORES  # 2048 rows per core
NBLK = W // 128    # 16

F32R = mybir.dt.float32r
F32 = mybir.dt.float32


# ---------- host-side construction of the reference linear map ----------
def _bit_reversal_indices(n):
    bits = int(np.log2(n))
    val = np.arange(n)
    src = np.zeros(n, dtype=np.int64)
    for _ in range(bits):
        src = (src << 1) | (val & 1)
        val = val >> 1
    return src


def _inter_stage_indices(n, stage_idx):
    prev = 2 ** stage_idx
    kb = np.arange(n // 2)
    g = kb // prev
    kw = kb % prev
    half = (kw >= prev // 2).astype(np.int64)
    off = kw % (prev // 2)
    nb = prev // 2
    sE = (2 * g) * nb + off
    sO = (2 * g + 1) * nb + off
    perm = np.empty(n, dtype=np.int64)
    perm[2 * kb] = 2 * sE + half
    perm[2 * kb + 1] = 2 * sO + half
    return perm


def _final_unscramble_indices(n):
    k = np.arange(n // 2)
    perm = np.empty(n, dtype=np.int64)
    perm[k] = 2 * k
    perm[k + n // 2] = 2 * k + 1
    return perm


def _butterfly_weights(n, stage_idx):
    Nb = 2 ** (stage_idx + 1)
    nub = Nb // 2
    reps = n // Nb
    k = np.arange(nub)
    rW = np.cos(2.0 * np.pi * k / Nb)
    iW = -np.sin(2.0 * np.pi * k / Nb)
    Wm = np.zeros((nub, 4, 4), dtype=np.float64)
    Wm[:, 0, 0] = 1.0; Wm[:, 0, 2] = rW;  Wm[:, 0, 3] = -iW
    Wm[:, 1, 1] = 1.0; Wm[:, 1, 2] = iW;  Wm[:, 1, 3] = rW
    Wm[:, 2, 0] = 1.0; Wm[:, 2, 2] = -rW; Wm[:, 2, 3] = iW
    Wm[:, 3, 1] = 1.0; Wm[:, 3, 2] = -iW; Wm[:, 3, 3] = -rW
    return np.tile(Wm, (reps, 1, 1))


def apply_pipeline(v2d):
    """Apply the reference linear pipeline to rows of v2d [batch, 2048] (numpy)."""
    b = v2d.shape[0]
    n = W // 2
    stages = int(np.log2(n))
    v = v2d.reshape(b, n, 2)
    v = v[:, _bit_reversal_indices(n), :]
    for s in range(stages):
        if s > 0:
            v = v[:, _inter_stage_indices(n, s), :]
        Wm = _butterfly_weights(n, s)
        g = v.reshape(b, n // 2, 4)
        g = np.einsum('goi,bgi->bgo', Wm, g)
        v = g.reshape(b, n, 2)
    v = v[:, _final_unscramble_indices(n), :]
    return v.reshape(b, W)


_MT_CACHE = None


def _build_mt():
    """Mt[e_in, e_out] such that out = x @ Mt."""
    global _MT_CACHE
    if _MT_CACHE is None:
        E = np.eye(W, dtype=np.float64)
        _MT_CACHE = np.ascontiguousarray(apply_pipeline(E).astype(np.float32))
    return _MT_CACHE


# ---------- bass kernel ----------
_NC_CACHE = None


def _build_bass():
    global _NC_CACHE
    if _NC_CACHE is not None:
        return _NC_CACHE
    nc = bacc.Bacc("TRN2", target_bir_lowering=False, debug=False)
    x = nc.dram_tensor("x", [RPC, W], F32R, kind="ExternalInput").ap()
    mt = nc.dram_tensor("mt", [128, NBLK * W], F32R, kind="ExternalInput").ap()
    iden = nc.dram_tensor("iden", [128, 128], F32R, kind="ExternalInput").ap()
    y = nc.dram_tensor("y", [RPC, W], F32, kind="ExternalOutput").ap()

    with tile.TileContext(nc) as tc:
        with tc.tile_pool(name="const", bufs=1) as cpool, \
             tc.tile_pool(name="mpool", bufs=1) as mpool, \
             tc.tile_pool(name="io", bufs=2) as iopool, \
             tc.tile_pool(name="xt", bufs=2) as xtpool, \
             tc.tile_pool(name="pst", bufs=2, space="PSUM") as pspool, \
             tc.tile_pool(name="psmm", bufs=1, space="PSUM") as mmpool:
            ident = cpool.tile([128, 128], F32R)
            nc.sync.dma_start(out=ident[:], in_=iden)
            mtall = mpool.tile([128, NBLK * W], F32R)
            nc.sync.dma_start(out=mtall[:], in_=mt)
            mts = [mtall[:, j * W:(j + 1) * W] for j in range(NBLK)]
            for t in range(RPC // 128):
                t0 = iopool.tile([128, W], F32R, tag="t0")
                nc.sync.dma_start(out=t0[:], in_=x[128 * t:128 * (t + 1), :])
                xts = []
                for j in range(NBLK):
                    pst = pspool.tile([128, 128], F32R, tag="pst")
                    nc.tensor.transpose(pst[:], t0[:, 128 * j:128 * (j + 1)], ident[:])
                    xtj = xtpool.tile([128, 128], F32R, tag=f"xt{j}")
                    nc.scalar.copy(xtj[:], pst[:])
                    xts.append(xtj)
                psn = [mmpool.tile([128, 512], F32, tag=f"mm{n}", name=f"mm{t}_{n}")
                       for n in range(4)]
                for j in range(NBLK):
                    for n in range(4):
                        nc.tensor.matmul(
                            psn[n][:], xts[j][:], mts[j][:, 512 * n:512 * (n + 1)],
                            start=(j == 0), stop=(j == NBLK - 1))
                o = iopool.tile([128, W], F32, tag="o")
                for n in range(4):
                    nc.scalar.copy(o[:, 512 * n:512 * (n + 1)], psn[n][:])
                nc.sync.dma_start(out=y[128 * t:128 * (t + 1), :], in_=o[:])
    nc.finalize()
    _NC_CACHE = nc
    return nc


def kernel(x, _trace=False, _trace_kwargs=None):
    x = np.asarray(x)
    orig_shape = x.shape
    xf = np.ascontiguousarray(x.reshape(B, W).astype(np.float32))
    mt = _build_mt()
    # [128 partitions, 16 blocks * 2048]: mtk[p, j*W+n] = Mt[128j+p, n]
    mtk = np.ascontiguousarray(
        mt.reshape(NBLK, 128, W).transpose(1, 0, 2).reshape(128, NBLK * W))
    nc = _build_bass()
    iden = np.eye(128, dtype=np.float32)
    in_maps = [
        {"x": np.ascontiguousarray(xf[c * RPC:(c + 1) * RPC]), "mt": mtk, "iden": iden}
        for c in range(NCORES)
    ]
    kw = {}
    if _trace:
        kw = dict(trace=True, **(_trace_kwargs or {}))
    res = run_bass_kernel_spmd(nc, in_maps, list(range(NCORES)), **kw)
    out = np.concatenate([r["y"] for r in res.results], axis=0)
    out = out.reshape(orig_shape).astype(np.float32)
    if _trace:
        return out, res
    return out
